# revision 19
# baseline (speedup 1.0000x reference)
"""CRF loss (sum of log-likelihoods) on 8 Trainium2 NeuronCores.

Problem: emissions (512, 8192, 7) f32, tags/mask (512, 8192), transition
params (7,)/(7,7). Output: scalar f32 total log-likelihood.

Strategy (data-parallel over batch, per the sharding hint):
  - 8 cores x 1024 batches. Batch on SBUF partitions: b = g*128 + p with
    g in [0,8) groups on the free dim.
  - Denominator (log-partition) via the forward algorithm in LINEAR space:
    P_s = (P_{s-1} @ exp(trans)) * exp(e_s), renormalized per batch every
    RENORM steps by its max (log of the scales accumulated, Ln'd in bulk at
    the end). Per step: one expand-mul [128, g*49], one grouped reduce, one
    emission mul [128, g*7] - all on VectorE; exp/log run in bulk on ScalarE.
  - Numerator: gold emissions e[s,b,tags[s,b]] gathered with a 3-round
    binary select tree (copy_predicated on bit masks of the tag), start/end
    transition gathers the same way; the tiny transition-pair-sum
    sum_s trans[t_s, t_{s+1}] is a 49-bin histogram dot done on host.
  - Each core returns per-partition partial sums [128, 2]; host adds them up.
"""

import sys

import numpy as np

for _p in ("/root/.axon_site/_ro/trn_rl_repo", "/opt/trn_rl_repo"):
    if _p not in sys.path:
        sys.path.append(_p)

S, B, T = 512, 8192, 7
NCORES = 8
BS = B // NCORES  # 1024 batches per core
PARTS = 128
RENORM = 16
SC = 64  # steps per emission chunk

# set by test harness to capture a profile
TRACE = False
LAST_EXEC_NS = None


def build_body(tc, o_ap, e_ap, tg_ap, cst_ap, *, s_len=S, bs=BS, sc=SC):
    """Emit the per-core kernel into TileContext `tc`.

    o_ap: DRAM out [128, 2] f32 (col0 = sum_g denom, col1 = numer partials)
    e_ap: DRAM in [s_len, bs, 7] f32 emissions shard
    tg_ap: DRAM in [128, s_len * g] f32 tags, layout [p, (s, g)]
    cst_ap: DRAM in [1, 81] f32 consts:
        [0:7]=exp(start) [8:15]=exp(end) [16:23]=start [24:31]=end
        [32:81]=ET[j, i] = exp(trans[i, j])
    """
    import concourse.bass as bass
    import concourse.mybir as mybir

    nc = tc.nc
    fp32 = mybir.dt.float32
    ALU = mybir.AluOpType
    ACTF = mybir.ActivationFunctionType
    G = bs // PARTS
    nchunk = s_len // sc
    n_renorm = (s_len - 1) // RENORM  # renorms at s = RENORM, 2*RENORM, ...
    CL = sc * G * T  # elems per partition per chunk

    singles = tc.alloc_tile_pool(name="singles", bufs=1)
    epool = tc.alloc_tile_pool(name="epool", bufs=2)
    state = tc.alloc_tile_pool(name="state", bufs=2)
    bitp = tc.alloc_tile_pool(name="bitp", bufs=2)

    csts = singles.tile([PARTS, 81], fp32)
    nc.sync.dma_start(out=csts, in_=cst_ap.to_broadcast((PARTS, 81)))
    tgb = singles.tile([PARTS, s_len * G], fp32)
    nc.sync.dma_start(out=tgb, in_=tg_ap)
    xbuf = singles.tile([PARTS, s_len, G, T], fp32)
    mlog = singles.tile([PARTS, n_renorm + 1, G], fp32)
    egp = singles.tile([PARTS, nchunk + 2], fp32)
    ou = singles.tile([PARTS, 2], fp32)

    # emissions DRAM view: [p, s, g, j]
    ev = e_ap.rearrange("s (g p) t -> p s g t", p=PARTS)

    ET = csts[:, 32:81].rearrange("p (j i) -> p j i", j=T)  # [128, 7, 7]

    def load_chunk(c):
        eb = epool.tile([PARTS, CL + T], fp32, tag="ebuf")
        nc.vector.memset(eb[:, CL : CL + T], 0.0)
        # 4 DMAs per chunk so several queues run in parallel
        q = sc // 4
        for k in range(4):
            s0 = c * sc + k * q
            nc.sync.dma_start(
                out=eb[:, k * q * G * T : (k + 1) * q * G * T].rearrange(
                    "p (s g t) -> p s g t", s=q, g=G
                ),
                in_=ev[:, s0 : s0 + q],
            )
        return eb

    def exp_chunk(c, eb):
        nc.scalar.activation(
            out=xbuf[:, c * sc : (c + 1) * sc].rearrange("p s g t -> p (s g t)"),
            in_=eb[:, 0:CL],
            func=ACTF.Exp,
        )

    def egold_chunk(c, eb):
        # all on GPSIMD (idle engine) with arithmetic selects
        # sel(b, t, f) = f + b * (t - f); tree rounds narrow 7 -> 4 -> 2 -> 1
        n = sc * G
        tgs = tgb[:, c * n : (c + 1) * n]
        b2 = bitp.tile([PARTS, n], fp32, tag="b2")
        t2 = bitp.tile([PARTS, n], fp32, tag="t2")
        b1 = bitp.tile([PARTS, n], fp32, tag="b1")
        b0 = bitp.tile([PARTS, n], fp32, tag="b0")
        gp = nc.gpsimd
        nc.vector.tensor_scalar(b2, tgs, 4.0, None, ALU.is_ge)
        nc.vector.scalar_tensor_tensor(t2, b2, -4.0, tgs, ALU.mult, ALU.add)
        nc.vector.tensor_scalar(b1, t2, 2.0, None, ALU.is_ge)
        nc.vector.scalar_tensor_tensor(b0, b1, -2.0, t2, ALU.mult, ALU.add)
        g7 = lambda off, w: eb[:, off : off + n * T].rearrange(
            "p (n c) -> p n c", c=T
        )[:, :, 0:w]
        bc = lambda b, w: b.unsqueeze(2).broadcast_to((PARTS, n, w))
        dif = bitp.tile([PARTS, n, 4], fp32, tag="dif")
        for b, w in ((b2, 4), (b1, 2), (b0, 1)):
            gp.tensor_tensor(
                dif[:, :, 0:w], g7(w, w), g7(0, w), ALU.subtract
            )
            gp.tensor_tensor(dif[:, :, 0:w], bc(b, w), dif[:, :, 0:w], ALU.mult)
            gp.tensor_tensor(g7(0, w), g7(0, w), dif[:, :, 0:w], ALU.add)
        nc.vector.tensor_reduce(
            egp[:, c : c + 1], g7(0, 1).rearrange("p n c -> p (n c)"),
            mybir.AxisListType.X, ALU.add,
        )

    def sel8(dst_col, toff, coff):
        """egp[:, dst_col] = sum_g table[coff][tg[:, toff + g]] (8-entry table)."""
        tcols = tgb[:, toff : toff + G]
        i32 = mybir.dt.int32
        sb2 = bitp.tile([PARTS, G], i32, tag="sb2")
        st2 = bitp.tile([PARTS, G], fp32, tag="st2")
        sb1 = bitp.tile([PARTS, G], i32, tag="sb1")
        sb0 = bitp.tile([PARTS, G], i32, tag="sb0")
        ssel5 = bitp.tile([PARTS, G, 5], fp32, tag="ssel")
        ssel = ssel5[:, :, 0:4]
        nc.vector.tensor_scalar(sb2, tcols, 4.0, None, ALU.is_ge)
        nc.vector.scalar_tensor_tensor(st2, sb2, -4.0, tcols, ALU.mult, ALU.add)
        nc.vector.tensor_scalar(sb1, st2, 2.0, None, ALU.is_ge)
        nc.vector.scalar_tensor_tensor(sb0, sb1, -2.0, st2, ALU.mult, ALU.add)
        cb = lambda off, w: csts[:, coff + off : coff + off + w].unsqueeze(1).broadcast_to((PARTS, G, w))
        bc = lambda b, w: b.unsqueeze(2).broadcast_to((PARTS, G, w))
        nc.vector.tensor_copy(ssel, cb(0, 4))
        nc.vector.copy_predicated(ssel, bc(sb2, 4), cb(4, 4))
        nc.vector.copy_predicated(ssel[:, :, 0:2], bc(sb1, 2), ssel[:, :, 2:4])
        nc.vector.copy_predicated(ssel[:, :, 0:1], bc(sb0, 1), ssel[:, :, 1:2])
        nc.vector.tensor_reduce(
            egp[:, dst_col : dst_col + 1],
            ssel[:, :, 0:1].rearrange("p g c -> p (g c)"),
            mybir.AxisListType.X, ALU.add,
        )

    # ---- pipeline ----
    eb_cur = load_chunk(0)
    exp_chunk(0, eb_cur)

    P = state.tile([PARTS, G, T], fp32, tag="P")
    nc.vector.tensor_mul(
        P, xbuf[:, 0],
        csts[:, 0:7].unsqueeze(1).broadcast_to((PARTS, G, T)),
    )

    ebs = {0: eb_cur}
    kre = 0
    for c in range(nchunk):
        if c + 1 < nchunk:
            ebs[c + 1] = load_chunk(c + 1)
            exp_chunk(c + 1, ebs[c + 1])
        s_lo = c * sc
        for s in range(max(s_lo, 1), s_lo + sc):
            if s % RENORM == 0:
                m = mlog[:, kre]
                nc.vector.tensor_reduce(m, P, mybir.AxisListType.X, ALU.max)
                rinv = state.tile([PARTS, G], fp32, tag="rinv")
                nc.vector.reciprocal(rinv, m)
                Pn = state.tile([PARTS, G, T], fp32, tag="P")
                nc.vector.tensor_mul(
                    Pn, P,
                    rinv.unsqueeze(2).broadcast_to((PARTS, G, T)),
                )
                P = Pn
                kre += 1
            r = state.tile([PARTS, G, T, T], fp32, tag="r")
            nc.vector.tensor_mul(
                r,
                P.unsqueeze(2).broadcast_to((PARTS, G, T, T)),
                ET.unsqueeze(1).broadcast_to((PARTS, G, T, T)),
            )
            q = state.tile([PARTS, G, T], fp32, tag="q")
            nc.vector.tensor_reduce(
                q.rearrange("p g j -> p (g j)"),
                r.rearrange("p g j i -> p (g j) i"),
                mybir.AxisListType.X, ALU.add,
            )
            Pn = state.tile([PARTS, G, T], fp32, tag="P")
            nc.vector.tensor_mul(Pn, q, xbuf[:, s])
            P = Pn
        # numerator work for this chunk (after the hot loop of the chunk)
        egold_chunk(c, ebs[c])
        del ebs[c]

    # ---- final combine ----
    zt = state.tile([PARTS, G, T], fp32, tag="r")
    nc.vector.tensor_mul(
        zt, P, csts[:, 8:15].unsqueeze(1).broadcast_to((PARTS, G, T))
    )
    nc.vector.tensor_reduce(mlog[:, n_renorm], zt, mybir.AxisListType.X, ALU.add)
    lnm = singles.tile([PARTS, n_renorm + 1, G], fp32)
    # scale into ScalarE Ln's valid input range; host adds back
    # (n_renorm + 1) * 32 * ln(2) per batch.
    nc.scalar.activation(
        out=lnm.rearrange("p k g -> p (k g)"),
        in_=mlog.rearrange("p k g -> p (k g)"),
        func=ACTF.Ln,
        scale=float(2.0**-32),
    )
    dg = state.tile([PARTS, G], fp32, tag="rinv")
    nc.vector.tensor_reduce(
        dg, lnm.rearrange("p k g -> p g k"), mybir.AxisListType.X, ALU.add
    )
    nc.vector.tensor_reduce(ou[:, 0:1], dg, mybir.AxisListType.X, ALU.add)

    sel8(nchunk, 0, 16)  # start_transitions[tags[0]]
    sel8(nchunk + 1, (s_len - 1) * G, 24)  # end_transitions[tags[-1]]
    nc.vector.tensor_reduce(ou[:, 1:2], egp, mybir.AxisListType.X, ALU.add)
    nc.sync.dma_start(out=o_ap, in_=ou)

    for pool in (bitp, state, epool, singles):
        pool.release()



def build_body2(tc, o_ap, d_ap, e_ap, tg_ap, cst_ap, bd_ap, selz_ap, rep_ap,
                *, s_len=S, bs=BS, sc=SC):
    """v2: transposed-state chain. State PT [56=(g,j), 128=p] in SBUF; the
    tag-mix + i-reduction is one PE matmul with a stationary block-diagonal
    exp(trans); VectorE does a single [56,128] multiply per step. Renorm by
    group-sums via selector matmuls. Numerator machinery identical to v1.
    """
    import concourse.mybir as mybir
    from concourse.masks import make_identity

    nc = tc.nc
    fp32 = mybir.dt.float32
    ALU = mybir.AluOpType
    ACTF = mybir.ActivationFunctionType
    G = bs // PARTS
    GJ = G * T  # 56 partitions for the transposed state
    nchunk = s_len // sc
    n_renorm = (s_len - 1) // RENORM
    CL = sc * G * T

    singles = tc.alloc_tile_pool(name="singles", bufs=1)
    epool = tc.alloc_tile_pool(name="epool", bufs=2)
    xpool = tc.alloc_tile_pool(name="xpool", bufs=2)
    state = tc.alloc_tile_pool(name="state", bufs=2)
    bitp = tc.alloc_tile_pool(name="bitp", bufs=2)
    ptp = tc.alloc_tile_pool(name="ptp", bufs=2, space="PSUM")
    pqp = tc.alloc_tile_pool(name="pqp", bufs=1, space="PSUM")
    prp = tc.alloc_tile_pool(name="prp", bufs=1, space="PSUM")

    csts = singles.tile([PARTS, 81], fp32)
    nc.sync.dma_start(out=csts, in_=cst_ap.to_broadcast((PARTS, 81)))
    tgb = singles.tile([PARTS, s_len * G], fp32)
    nc.sync.dma_start(out=tgb, in_=tg_ap)
    bdt = singles.tile([GJ, GJ], fp32)
    nc.sync.dma_start(out=bdt, in_=bd_ap)
    selz = singles.tile([GJ, 17], fp32)
    nc.sync.dma_start(out=selz, in_=selz_ap)
    rept = singles.tile([G, GJ], fp32)
    nc.sync.dma_start(out=rept, in_=rep_ap)
    eye = singles.tile([PARTS, PARTS], fp32)
    make_identity(nc, eye)

    mlog = singles.tile([G, n_renorm + 1, PARTS], fp32)
    egp = singles.tile([PARTS, nchunk + 2], fp32)
    ou = singles.tile([PARTS, 2], fp32)
    nc.vector.memset(ou[:, 0:1], 0.0)

    ev = e_ap.rearrange("s (g p) t -> p s g t", p=PARTS)

    def load_chunk(c):
        eb = epool.tile([PARTS, CL + T], fp32, tag="ebuf")
        nc.vector.memset(eb[:, CL : CL + T], 0.0)
        q = sc // 4
        for k in range(4):
            s0 = c * sc + k * q
            nc.sync.dma_start(
                out=eb[:, k * q * G * T : (k + 1) * q * G * T].rearrange(
                    "p (s g t) -> p s g t", s=q, g=G
                ),
                in_=ev[:, s0 : s0 + q],
            )
        return eb

    def exp_chunk(eb):
        xb = xpool.tile([PARTS, CL], fp32, tag="xb")
        nc.scalar.activation(out=xb, in_=eb[:, 0:CL], func=ACTF.Exp)
        return xb

    def new_xt():
        xt = xpool.tile([GJ, sc * PARTS], fp32, tag="xt")
        return xt

    def build_xt_step(xb, xt, sl):
        tp = ptp.tile([GJ, PARTS], fp32, tag="tp")
        nc.tensor.transpose(tp, xb[:, sl * GJ : (sl + 1) * GJ], eye)
        nc.scalar.copy(out=xt[:, sl * PARTS : (sl + 1) * PARTS], in_=tp)

    def egold_chunk(c, eb):
        # all on GPSIMD (idle engine) with arithmetic selects
        # sel(b, t, f) = f + b * (t - f); tree rounds narrow 7 -> 4 -> 2 -> 1
        n = sc * G
        tgs = tgb[:, c * n : (c + 1) * n]
        b2 = bitp.tile([PARTS, n], fp32, tag="b2")
        t2 = bitp.tile([PARTS, n], fp32, tag="t2")
        b1 = bitp.tile([PARTS, n], fp32, tag="b1")
        b0 = bitp.tile([PARTS, n], fp32, tag="b0")
        gp = nc.gpsimd
        nc.vector.tensor_scalar(b2, tgs, 4.0, None, ALU.is_ge)
        nc.vector.scalar_tensor_tensor(t2, b2, -4.0, tgs, ALU.mult, ALU.add)
        nc.vector.tensor_scalar(b1, t2, 2.0, None, ALU.is_ge)
        nc.vector.scalar_tensor_tensor(b0, b1, -2.0, t2, ALU.mult, ALU.add)
        g7 = lambda off, w: eb[:, off : off + n * T].rearrange(
            "p (n c) -> p n c", c=T
        )[:, :, 0:w]
        bc = lambda b, w: b.unsqueeze(2).broadcast_to((PARTS, n, w))
        dif = bitp.tile([PARTS, n, 4], fp32, tag="dif")
        for b, w in ((b2, 4), (b1, 2), (b0, 1)):
            gp.tensor_tensor(
                dif[:, :, 0:w], g7(w, w), g7(0, w), ALU.subtract
            )
            gp.tensor_tensor(dif[:, :, 0:w], bc(b, w), dif[:, :, 0:w], ALU.mult)
            gp.tensor_tensor(g7(0, w), g7(0, w), dif[:, :, 0:w], ALU.add)
        nc.vector.tensor_reduce(
            egp[:, c : c + 1], g7(0, 1).rearrange("p n c -> p (n c)"),
            mybir.AxisListType.X, ALU.add,
        )

    def sel8(dst_col, toff, coff):
        tcols = tgb[:, toff : toff + G]
        i32 = mybir.dt.int32
        sb2 = bitp.tile([PARTS, G], i32, tag="sb2")
        st2 = bitp.tile([PARTS, G], fp32, tag="st2")
        sb1 = bitp.tile([PARTS, G], i32, tag="sb1")
        sb0 = bitp.tile([PARTS, G], i32, tag="sb0")
        ssel5 = bitp.tile([PARTS, G, 5], fp32, tag="ssel")
        ssel = ssel5[:, :, 0:4]
        nc.vector.tensor_scalar(sb2, tcols, 4.0, None, ALU.is_ge)
        nc.vector.scalar_tensor_tensor(st2, sb2, -4.0, tcols, ALU.mult, ALU.add)
        nc.vector.tensor_scalar(sb1, st2, 2.0, None, ALU.is_ge)
        nc.vector.scalar_tensor_tensor(sb0, sb1, -2.0, st2, ALU.mult, ALU.add)
        cb = lambda off, w: csts[
            :, coff + off : coff + off + w
        ].unsqueeze(1).broadcast_to((PARTS, G, w))
        bc = lambda b, w: b.unsqueeze(2).broadcast_to((PARTS, G, w))
        nc.vector.tensor_copy(ssel, cb(0, 4))
        nc.vector.copy_predicated(ssel, bc(sb2, 4), cb(4, 4))
        nc.vector.copy_predicated(ssel[:, :, 0:2], bc(sb1, 2), ssel[:, :, 2:4])
        nc.vector.copy_predicated(ssel[:, :, 0:1], bc(sb0, 1), ssel[:, :, 1:2])
        nc.vector.tensor_reduce(
            egp[:, dst_col : dst_col + 1],
            ssel[:, :, 0:1].rearrange("p g c -> p (g c)"),
            mybir.AxisListType.X, ALU.add,
        )

    # ---- prologue: chunk 0 fully staged ----
    eb_cur = load_chunk(0)
    xb_cur = exp_chunk(eb_cur)
    xt_cur = new_xt()
    for sl in range(sc):
        build_xt_step(xb_cur, xt_cur, sl)

    # two independent half-chains (batches split along the free dim) so the
    # PE matmul of one half overlaps the VectorE multiply of the other
    H = PARTS // 2
    PTh = [None, None]
    for h in range(2):
        PTx = state.tile([GJ, H], fp32, tag=f"PT{h}")
        nc.vector.tensor_scalar_mul(
            PTx, xt_cur[:, h * H : h * H + H], selz[:, 16:17]
        )
        PTh[h] = PTx

    kre = 0
    ebs = {0: eb_cur}
    for c in range(nchunk):
        have_next = c + 1 < nchunk
        if have_next:
            ebs[c + 1] = load_chunk(c + 1)
            xb_next = exp_chunk(ebs[c + 1])
            xt_next = new_xt()
        s_lo = c * sc
        if c == 0 and have_next:
            build_xt_step(xb_next, xt_next, 0)  # s-loop below skips s=0
        for s in range(max(s_lo, 1), s_lo + sc):
            sl = s - s_lo
            if s % RENORM == 0:
                mgs = []
                for h in range(2):
                    mg = prp.tile([G, H], fp32, tag=f"mg{h}")
                    nc.tensor.matmul(
                        mg, selz[:, 0:G], PTh[h], start=True, stop=True
                    )
                    nc.scalar.copy(
                        out=mlog[:, kre, h * H : h * H + H], in_=mg
                    )
                    mgs.append(mg)
                for h in range(2):
                    rinv = state.tile([G, H], fp32, tag=f"rinv{h}")
                    nc.vector.reciprocal(rinv, mgs[h])
                    repm = prp.tile([GJ, H], fp32, tag=f"repm{h}")
                    nc.tensor.matmul(repm, rept, rinv, start=True, stop=True)
                    PTn = state.tile([GJ, H], fp32, tag=f"PT{h}")
                    nc.vector.tensor_mul(PTn, PTh[h], repm)
                    PTh[h] = PTn
                kre += 1
            qTs = []
            for h in range(2):
                qT = pqp.tile([GJ, H], fp32, tag=f"qT{h}")
                nc.tensor.matmul(qT, bdt, PTh[h], start=True, stop=True)
                qTs.append(qT)
            for h in range(2):
                PTn = state.tile([GJ, H], fp32, tag=f"PT{h}")
                nc.vector.tensor_mul(
                    PTn, qTs[h], xt_cur[:, sl * PARTS + h * H : sl * PARTS + h * H + H]
                )
                PTh[h] = PTn
            if have_next:
                build_xt_step(xb_next, xt_next, sl)
        egold_chunk(c, ebs[c])
        del ebs[c]
        if have_next:
            xb_cur, xt_cur = xb_next, xt_next

    # ---- final combine ----
    for h in range(2):
        zf = prp.tile([G, H], fp32, tag=f"mg{h}")
        nc.tensor.matmul(zf, selz[:, G : 2 * G], PTh[h], start=True, stop=True)
        nc.scalar.copy(out=mlog[:, n_renorm, h * H : h * H + H], in_=zf)
    lnm = singles.tile([G, n_renorm + 1, PARTS], fp32)
    nc.scalar.activation(
        out=lnm.rearrange("p k b -> p (k b)"),
        in_=mlog.rearrange("p k b -> p (k b)"),
        func=ACTF.Ln,
        scale=float(2.0**-32),
    )
    denb = singles.tile([G, PARTS], fp32)
    nc.vector.tensor_reduce(
        denb, lnm.rearrange("p k b -> p b k"), mybir.AxisListType.X, ALU.add
    )
    nc.sync.dma_start(out=d_ap, in_=denb)

    sel8(nchunk, 0, 16)
    sel8(nchunk + 1, (s_len - 1) * G, 24)
    nc.vector.tensor_reduce(ou[:, 1:2], egp, mybir.AxisListType.X, ALU.add)
    nc.sync.dma_start(out=o_ap, in_=ou)

    for pool in (prp, pqp, ptp, bitp, state, xpool, epool, singles):
        pool.release()


def make_v2_consts(start, end, trans):
    ET = np.exp(trans).astype(np.float32)  # [i, j]
    bd = np.zeros((56, 56), np.float32)
    for g in range(8):
        bd[g * 7 : (g + 1) * 7, g * 7 : (g + 1) * 7] = ET
    selz = np.zeros((56, 17), np.float32)
    rep = np.zeros((8, 56), np.float32)
    for g in range(8):
        for j in range(7):
            selz[g * 7 + j, g] = 1.0
            selz[g * 7 + j, 8 + g] = np.exp(end[j])
            selz[g * 7 + j, 16] = np.exp(start[j])
            rep[g, g * 7 + j] = 1.0
    return bd, selz, rep


_cache = {}


def get_compiled(s_len=S, bs=BS, sc=SC, variant=2):
    key = (s_len, bs, sc, variant)
    if key in _cache:
        return _cache[key]
    import concourse.bacc as bacc
    import concourse.mybir as mybir
    import concourse.tile as tile

    nc = bacc.Bacc(
        "TRN2", target_bir_lowering=False, debug=False, num_devices=NCORES
    )
    fp32 = mybir.dt.float32
    G = bs // PARTS
    e_d = nc.dram_tensor("e", [s_len, bs, T], fp32, kind="ExternalInput").ap()
    tg_d = nc.dram_tensor("tg", [PARTS, s_len * G], fp32, kind="ExternalInput").ap()
    cst_d = nc.dram_tensor("cst", [1, 81], fp32, kind="ExternalInput").ap()
    o_d = nc.dram_tensor("o", [PARTS, 2], fp32, kind="ExternalOutput").ap()
    if variant == 2:
        bd_d = nc.dram_tensor("bd", [56, 56], fp32, kind="ExternalInput").ap()
        selz_d = nc.dram_tensor("selz", [56, 17], fp32, kind="ExternalInput").ap()
        rep_d = nc.dram_tensor("rep", [8, 56], fp32, kind="ExternalInput").ap()
        d_d = nc.dram_tensor("d", [G, PARTS], fp32, kind="ExternalOutput").ap()
        with tile.TileContext(nc) as tc:
            build_body2(
                tc, o_d, d_d, e_d, tg_d, cst_d, bd_d, selz_d, rep_d,
                s_len=s_len, bs=bs, sc=sc,
            )
    else:
        with tile.TileContext(nc) as tc:
            build_body(tc, o_d, e_d, tg_d, cst_d, s_len=s_len, bs=bs, sc=sc)
    nc.compile()
    _cache[key] = nc
    return nc


def make_consts(start, end, trans):
    cst = np.zeros((1, 81), np.float32)
    cst[0, 0:7] = np.exp(start)
    cst[0, 8:15] = np.exp(end)
    cst[0, 16:23] = start
    cst[0, 24:31] = end
    cst[0, 32:81] = np.exp(trans).T.ravel()  # ET[j, i] = exp(trans[i, j])
    return cst


def _numpy_fallback(emissions, start, end, trans, tags, mask):
    maskf = mask.astype(np.float64)
    e = emissions.astype(np.float64)
    s_len, batch = tags.shape
    emit = np.take_along_axis(e, tags[:, :, None], axis=2)[..., 0]
    trans_sc = trans[tags[:-1], tags[1:]].astype(np.float64)
    num = start[tags[0]].astype(np.float64) + emit[0]
    num = num + ((trans_sc + emit[1:]) * maskf[1:]).sum(axis=0)
    seq_ends = mask.astype(np.int64).sum(axis=0) - 1
    last_tags = tags[seq_ends, np.arange(batch)]
    num = num + end[last_tags]
    score = start[None, :] + e[0]
    for i in range(1, s_len):
        nxt = score[:, :, None] + trans[None] + e[i][:, None, :]
        mx = nxt.max(axis=1)
        nxt = mx + np.log(np.exp(nxt - mx[:, None, :]).sum(axis=1))
        score = np.where(mask[i][:, None], nxt, score)
    mx = (score + end[None, :]).max(axis=1)
    denom = mx + np.log(np.exp(score + end[None, :] - mx[:, None]).sum(axis=1))
    return np.float32((num - denom).sum())


def kernel(emissions, start_transitions, end_transitions, transitions, tags, mask):
    global LAST_EXEC_NS
    emissions = np.asarray(emissions, np.float32)
    start = np.asarray(start_transitions, np.float32)
    end = np.asarray(end_transitions, np.float32)
    trans = np.asarray(transitions, np.float32)
    tags = np.asarray(tags)
    mask_np = np.asarray(mask)

    if not mask_np.all():
        return _numpy_fallback(
            emissions, start, end, trans, tags.astype(np.int64), mask_np
        )

    from concourse import bass_utils

    variant = 2
    nc = get_compiled(variant=variant)
    cst = make_consts(start, end, trans)
    tags32 = tags.astype(np.int32)
    in_maps = []
    G = BS // PARTS
    if variant == 2:
        bd, selz, rep = make_v2_consts(start, end, trans)
    for c in range(NCORES):
        sl = slice(c * BS, (c + 1) * BS)
        e_sh = np.ascontiguousarray(emissions[:, sl, :])
        tgc = (
            tags32[:, sl]
            .reshape(S, G, PARTS)
            .transpose(2, 0, 1)
            .reshape(PARTS, S * G)
            .astype(np.float32)
        )
        m = {"e": e_sh, "tg": np.ascontiguousarray(tgc), "cst": cst}
        if variant == 2:
            m.update({"bd": bd, "selz": selz, "rep": rep})
        in_maps.append(m)

    trace = TRACE
    if trace:
        try:
            from antenv.axon_hooks import get_axon_ntff_profile_hook  # noqa: F401
        except ImportError:
            trace = False
    res = bass_utils.run_bass_kernel_spmd(
        nc, in_maps, core_ids=list(range(NCORES)), trace=trace
    )
    LAST_EXEC_NS = res.exec_time_ns

    total = 0.0
    for c in range(NCORES):
        o = res.results[c]["o"].astype(np.float64)
        total += o[:, 1].sum() - o[:, 0].sum()
        if variant == 2:
            total -= res.results[c]["d"].astype(np.float64).sum()
    # Ln-scale correction: device computed ln(m * 2^-32) per mlog slot
    n_renorm = (S - 1) // RENORM
    total -= B * (n_renorm + 1) * 32.0 * np.log(2.0)

    # host part: sum_s trans[t_s, t_{s+1}] via 49-bin histogram
    codes = (7 * tags32[:-1] + tags32[1:]).ravel()
    cnt = np.bincount(codes, minlength=49).astype(np.float64)
    total += float(cnt @ trans.astype(np.float64).ravel())
    return np.float32(total)


# revision 20
# speedup vs baseline: 1.1626x; 1.1626x over previous
"""CRF loss (sum of log-likelihoods) on 8 Trainium2 NeuronCores.

Problem: emissions (512, 8192, 7) f32, tags/mask (512, 8192), transition
params (7,)/(7,7). Output: scalar f32 total log-likelihood.

Strategy (data-parallel over batch, per the sharding hint):
  - 8 cores x 1024 batches. Batch on SBUF partitions: b = g*128 + p with
    g in [0,8) groups on the free dim.
  - Denominator (log-partition) via the forward algorithm in LINEAR space:
    P_s = (P_{s-1} @ exp(trans)) * exp(e_s), renormalized per batch every
    RENORM steps by its max (log of the scales accumulated, Ln'd in bulk at
    the end). Per step: one expand-mul [128, g*49], one grouped reduce, one
    emission mul [128, g*7] - all on VectorE; exp/log run in bulk on ScalarE.
  - Numerator: gold emissions e[s,b,tags[s,b]] gathered with a 3-round
    binary select tree (copy_predicated on bit masks of the tag), start/end
    transition gathers the same way; the tiny transition-pair-sum
    sum_s trans[t_s, t_{s+1}] is a 49-bin histogram dot done on host.
  - Each core returns per-partition partial sums [128, 2]; host adds them up.
"""

import sys

import numpy as np

for _p in ("/root/.axon_site/_ro/trn_rl_repo", "/opt/trn_rl_repo"):
    if _p not in sys.path:
        sys.path.append(_p)

S, B, T = 512, 8192, 7
NCORES = 8
BS = B // NCORES  # 1024 batches per core
PARTS = 128
RENORM = 16
SC = 64  # steps per emission chunk

# set by test harness to capture a profile
TRACE = False
LAST_EXEC_NS = None


def build_body(tc, o_ap, e_ap, tg_ap, cst_ap, *, s_len=S, bs=BS, sc=SC):
    """Emit the per-core kernel into TileContext `tc`.

    o_ap: DRAM out [128, 2] f32 (col0 = sum_g denom, col1 = numer partials)
    e_ap: DRAM in [s_len, bs, 7] f32 emissions shard
    tg_ap: DRAM in [128, s_len * g] f32 tags, layout [p, (s, g)]
    cst_ap: DRAM in [1, 81] f32 consts:
        [0:7]=exp(start) [8:15]=exp(end) [16:23]=start [24:31]=end
        [32:81]=ET[j, i] = exp(trans[i, j])
    """
    import concourse.bass as bass
    import concourse.mybir as mybir

    nc = tc.nc
    fp32 = mybir.dt.float32
    ALU = mybir.AluOpType
    ACTF = mybir.ActivationFunctionType
    G = bs // PARTS
    nchunk = s_len // sc
    n_renorm = (s_len - 1) // RENORM  # renorms at s = RENORM, 2*RENORM, ...
    CL = sc * G * T  # elems per partition per chunk

    singles = tc.alloc_tile_pool(name="singles", bufs=1)
    epool = tc.alloc_tile_pool(name="epool", bufs=2)
    state = tc.alloc_tile_pool(name="state", bufs=2)
    bitp = tc.alloc_tile_pool(name="bitp", bufs=2)

    csts = singles.tile([PARTS, 81], fp32)
    nc.sync.dma_start(out=csts, in_=cst_ap.to_broadcast((PARTS, 81)))
    tgb = singles.tile([PARTS, s_len * G], fp32)
    nc.sync.dma_start(out=tgb, in_=tg_ap)
    xbuf = singles.tile([PARTS, s_len, G, T], fp32)
    mlog = singles.tile([PARTS, n_renorm + 1, G], fp32)
    egp = singles.tile([PARTS, nchunk + 2], fp32)
    ou = singles.tile([PARTS, 2], fp32)

    # emissions DRAM view: [p, s, g, j]
    ev = e_ap.rearrange("s (g p) t -> p s g t", p=PARTS)

    ET = csts[:, 32:81].rearrange("p (j i) -> p j i", j=T)  # [128, 7, 7]

    def load_chunk(c):
        eb = epool.tile([PARTS, CL + T], fp32, tag="ebuf")
        nc.vector.memset(eb[:, CL : CL + T], 0.0)
        # 4 DMAs per chunk so several queues run in parallel
        q = sc // 4
        for k in range(4):
            s0 = c * sc + k * q
            nc.sync.dma_start(
                out=eb[:, k * q * G * T : (k + 1) * q * G * T].rearrange(
                    "p (s g t) -> p s g t", s=q, g=G
                ),
                in_=ev[:, s0 : s0 + q],
            )
        return eb

    def exp_chunk(c, eb):
        nc.scalar.activation(
            out=xbuf[:, c * sc : (c + 1) * sc].rearrange("p s g t -> p (s g t)"),
            in_=eb[:, 0:CL],
            func=ACTF.Exp,
        )

    def egold_chunk(c, eb):
        n = sc * G
        tgs = tgb[:, c * n : (c + 1) * n]
        i32 = mybir.dt.int32
        b2 = bitp.tile([PARTS, n], i32, tag="b2")
        t2 = bitp.tile([PARTS, n], fp32, tag="t2")
        b1 = bitp.tile([PARTS, n], i32, tag="b1")
        b0 = bitp.tile([PARTS, n], i32, tag="b0")
        nc.vector.tensor_scalar(b2, tgs, 4.0, None, ALU.is_ge)
        nc.vector.scalar_tensor_tensor(t2, b2, -4.0, tgs, ALU.mult, ALU.add)
        nc.vector.tensor_scalar(b1, t2, 2.0, None, ALU.is_ge)
        nc.vector.scalar_tensor_tensor(b0, b1, -2.0, t2, ALU.mult, ALU.add)
        g7 = lambda off, w: eb[:, off : off + n * T].rearrange(
            "p (n c) -> p n c", c=T
        )[:, :, 0:w]
        bc = lambda b, w: b.unsqueeze(2).broadcast_to((PARTS, n, w))
        nc.vector.copy_predicated(g7(0, 4), bc(b2, 4), g7(4, 4))
        nc.vector.copy_predicated(g7(0, 2), bc(b1, 2), g7(2, 2))
        nc.vector.copy_predicated(g7(0, 1), bc(b0, 1), g7(1, 1))
        nc.vector.tensor_reduce(
            egp[:, c : c + 1], g7(0, 1).rearrange("p n c -> p (n c)"),
            mybir.AxisListType.X, ALU.add,
        )

    def sel8(dst_col, toff, coff):
        """egp[:, dst_col] = sum_g table[coff][tg[:, toff + g]] (8-entry table)."""
        tcols = tgb[:, toff : toff + G]
        i32 = mybir.dt.int32
        sb2 = bitp.tile([PARTS, G], i32, tag="sb2")
        st2 = bitp.tile([PARTS, G], fp32, tag="st2")
        sb1 = bitp.tile([PARTS, G], i32, tag="sb1")
        sb0 = bitp.tile([PARTS, G], i32, tag="sb0")
        ssel5 = bitp.tile([PARTS, G, 5], fp32, tag="ssel")
        ssel = ssel5[:, :, 0:4]
        nc.vector.tensor_scalar(sb2, tcols, 4.0, None, ALU.is_ge)
        nc.vector.scalar_tensor_tensor(st2, sb2, -4.0, tcols, ALU.mult, ALU.add)
        nc.vector.tensor_scalar(sb1, st2, 2.0, None, ALU.is_ge)
        nc.vector.scalar_tensor_tensor(sb0, sb1, -2.0, st2, ALU.mult, ALU.add)
        cb = lambda off, w: csts[:, coff + off : coff + off + w].unsqueeze(1).broadcast_to((PARTS, G, w))
        bc = lambda b, w: b.unsqueeze(2).broadcast_to((PARTS, G, w))
        nc.vector.tensor_copy(ssel, cb(0, 4))
        nc.vector.copy_predicated(ssel, bc(sb2, 4), cb(4, 4))
        nc.vector.copy_predicated(ssel[:, :, 0:2], bc(sb1, 2), ssel[:, :, 2:4])
        nc.vector.copy_predicated(ssel[:, :, 0:1], bc(sb0, 1), ssel[:, :, 1:2])
        nc.vector.tensor_reduce(
            egp[:, dst_col : dst_col + 1],
            ssel[:, :, 0:1].rearrange("p g c -> p (g c)"),
            mybir.AxisListType.X, ALU.add,
        )

    # ---- pipeline ----
    eb_cur = load_chunk(0)
    exp_chunk(0, eb_cur)

    P = state.tile([PARTS, G, T], fp32, tag="P")
    nc.vector.tensor_mul(
        P, xbuf[:, 0],
        csts[:, 0:7].unsqueeze(1).broadcast_to((PARTS, G, T)),
    )

    ebs = {0: eb_cur}
    kre = 0
    for c in range(nchunk):
        if c + 1 < nchunk:
            ebs[c + 1] = load_chunk(c + 1)
            exp_chunk(c + 1, ebs[c + 1])
        s_lo = c * sc
        for s in range(max(s_lo, 1), s_lo + sc):
            if s % RENORM == 0:
                m = mlog[:, kre]
                nc.vector.tensor_reduce(m, P, mybir.AxisListType.X, ALU.max)
                rinv = state.tile([PARTS, G], fp32, tag="rinv")
                nc.vector.reciprocal(rinv, m)
                Pn = state.tile([PARTS, G, T], fp32, tag="P")
                nc.vector.tensor_mul(
                    Pn, P,
                    rinv.unsqueeze(2).broadcast_to((PARTS, G, T)),
                )
                P = Pn
                kre += 1
            r = state.tile([PARTS, G, T, T], fp32, tag="r")
            nc.vector.tensor_mul(
                r,
                P.unsqueeze(2).broadcast_to((PARTS, G, T, T)),
                ET.unsqueeze(1).broadcast_to((PARTS, G, T, T)),
            )
            q = state.tile([PARTS, G, T], fp32, tag="q")
            nc.vector.tensor_reduce(
                q.rearrange("p g j -> p (g j)"),
                r.rearrange("p g j i -> p (g j) i"),
                mybir.AxisListType.X, ALU.add,
            )
            Pn = state.tile([PARTS, G, T], fp32, tag="P")
            nc.vector.tensor_mul(Pn, q, xbuf[:, s])
            P = Pn
        # numerator work for this chunk (after the hot loop of the chunk)
        egold_chunk(c, ebs[c])
        del ebs[c]

    # ---- final combine ----
    zt = state.tile([PARTS, G, T], fp32, tag="r")
    nc.vector.tensor_mul(
        zt, P, csts[:, 8:15].unsqueeze(1).broadcast_to((PARTS, G, T))
    )
    nc.vector.tensor_reduce(mlog[:, n_renorm], zt, mybir.AxisListType.X, ALU.add)
    lnm = singles.tile([PARTS, n_renorm + 1, G], fp32)
    # scale into ScalarE Ln's valid input range; host adds back
    # (n_renorm + 1) * 32 * ln(2) per batch.
    nc.scalar.activation(
        out=lnm.rearrange("p k g -> p (k g)"),
        in_=mlog.rearrange("p k g -> p (k g)"),
        func=ACTF.Ln,
        scale=float(2.0**-32),
    )
    dg = state.tile([PARTS, G], fp32, tag="rinv")
    nc.vector.tensor_reduce(
        dg, lnm.rearrange("p k g -> p g k"), mybir.AxisListType.X, ALU.add
    )
    nc.vector.tensor_reduce(ou[:, 0:1], dg, mybir.AxisListType.X, ALU.add)

    sel8(nchunk, 0, 16)  # start_transitions[tags[0]]
    sel8(nchunk + 1, (s_len - 1) * G, 24)  # end_transitions[tags[-1]]
    nc.vector.tensor_reduce(ou[:, 1:2], egp, mybir.AxisListType.X, ALU.add)
    nc.sync.dma_start(out=o_ap, in_=ou)

    for pool in (bitp, state, epool, singles):
        pool.release()



def build_body2(tc, o_ap, d_ap, e_ap, tg_ap, cst_ap, bd_ap, selz_ap, rep_ap,
                *, s_len=S, bs=BS, sc=SC):
    """v2: transposed-state chain. State PT [56=(g,j), 128=p] in SBUF; the
    tag-mix + i-reduction is one PE matmul with a stationary block-diagonal
    exp(trans); VectorE does a single [56,128] multiply per step. Renorm by
    group-sums via selector matmuls. Numerator machinery identical to v1.
    """
    import concourse.mybir as mybir
    from concourse.masks import make_identity

    nc = tc.nc
    fp32 = mybir.dt.float32
    ALU = mybir.AluOpType
    ACTF = mybir.ActivationFunctionType
    G = bs // PARTS
    GJ = G * T  # 56 partitions for the transposed state
    nchunk = s_len // sc
    n_renorm = (s_len - 1) // RENORM
    CL = sc * G * T

    singles = tc.alloc_tile_pool(name="singles", bufs=1)
    epool = tc.alloc_tile_pool(name="epool", bufs=2)
    xpool = tc.alloc_tile_pool(name="xpool", bufs=2)
    state = tc.alloc_tile_pool(name="state", bufs=2)
    bitp = tc.alloc_tile_pool(name="bitp", bufs=2)
    ptp = tc.alloc_tile_pool(name="ptp", bufs=2, space="PSUM")
    pqp = tc.alloc_tile_pool(name="pqp", bufs=1, space="PSUM")
    prp = tc.alloc_tile_pool(name="prp", bufs=1, space="PSUM")

    csts = singles.tile([PARTS, 81], fp32)
    nc.sync.dma_start(out=csts, in_=cst_ap.to_broadcast((PARTS, 81)))
    tgb = singles.tile([PARTS, s_len * G], fp32)
    nc.sync.dma_start(out=tgb, in_=tg_ap)
    bdt = singles.tile([GJ, GJ], fp32)
    nc.sync.dma_start(out=bdt, in_=bd_ap)
    selz = singles.tile([GJ, 17], fp32)
    nc.sync.dma_start(out=selz, in_=selz_ap)
    rept = singles.tile([G, GJ], fp32)
    nc.sync.dma_start(out=rept, in_=rep_ap)
    eye = singles.tile([PARTS, PARTS], fp32)
    make_identity(nc, eye)

    mlog = singles.tile([G, n_renorm + 1, PARTS], fp32)
    egp = singles.tile([PARTS, nchunk + 2], fp32)
    ou = singles.tile([PARTS, 2], fp32)
    nc.vector.memset(ou[:, 0:1], 0.0)

    ev = e_ap.rearrange("s (g p) t -> p s g t", p=PARTS)

    def load_chunk(c):
        eb = epool.tile([PARTS, CL + T], fp32, tag="ebuf")
        nc.vector.memset(eb[:, CL : CL + T], 0.0)
        q = sc // 4
        for k in range(4):
            s0 = c * sc + k * q
            nc.sync.dma_start(
                out=eb[:, k * q * G * T : (k + 1) * q * G * T].rearrange(
                    "p (s g t) -> p s g t", s=q, g=G
                ),
                in_=ev[:, s0 : s0 + q],
            )
        return eb

    def exp_chunk(eb):
        xb = xpool.tile([PARTS, CL], fp32, tag="xb")
        nc.scalar.activation(out=xb, in_=eb[:, 0:CL], func=ACTF.Exp)
        return xb

    def new_xt():
        xt = xpool.tile([GJ, sc * PARTS], fp32, tag="xt")
        return xt

    def build_xt_step(xb, xt, sl):
        tp = ptp.tile([GJ, PARTS], fp32, tag="tp")
        nc.tensor.transpose(tp, xb[:, sl * GJ : (sl + 1) * GJ], eye)
        nc.scalar.copy(out=xt[:, sl * PARTS : (sl + 1) * PARTS], in_=tp)

    def egold_chunk(c, eb):
        n = sc * G
        tgs = tgb[:, c * n : (c + 1) * n]
        i32 = mybir.dt.int32
        b2 = bitp.tile([PARTS, n], i32, tag="b2")
        t2 = bitp.tile([PARTS, n], fp32, tag="t2")
        b1 = bitp.tile([PARTS, n], i32, tag="b1")
        b0 = bitp.tile([PARTS, n], i32, tag="b0")
        nc.vector.tensor_scalar(b2, tgs, 4.0, None, ALU.is_ge)
        nc.vector.scalar_tensor_tensor(t2, b2, -4.0, tgs, ALU.mult, ALU.add)
        nc.vector.tensor_scalar(b1, t2, 2.0, None, ALU.is_ge)
        nc.vector.scalar_tensor_tensor(b0, b1, -2.0, t2, ALU.mult, ALU.add)
        g7 = lambda off, w: eb[:, off : off + n * T].rearrange(
            "p (n c) -> p n c", c=T
        )[:, :, 0:w]
        bc = lambda b, w: b.unsqueeze(2).broadcast_to((PARTS, n, w))
        nc.vector.copy_predicated(g7(0, 4), bc(b2, 4), g7(4, 4))
        nc.vector.copy_predicated(g7(0, 2), bc(b1, 2), g7(2, 2))
        nc.vector.copy_predicated(g7(0, 1), bc(b0, 1), g7(1, 1))
        nc.vector.tensor_reduce(
            egp[:, c : c + 1], g7(0, 1).rearrange("p n c -> p (n c)"),
            mybir.AxisListType.X, ALU.add,
        )

    def sel8(dst_col, toff, coff):
        tcols = tgb[:, toff : toff + G]
        i32 = mybir.dt.int32
        sb2 = bitp.tile([PARTS, G], i32, tag="sb2")
        st2 = bitp.tile([PARTS, G], fp32, tag="st2")
        sb1 = bitp.tile([PARTS, G], i32, tag="sb1")
        sb0 = bitp.tile([PARTS, G], i32, tag="sb0")
        ssel5 = bitp.tile([PARTS, G, 5], fp32, tag="ssel")
        ssel = ssel5[:, :, 0:4]
        nc.vector.tensor_scalar(sb2, tcols, 4.0, None, ALU.is_ge)
        nc.vector.scalar_tensor_tensor(st2, sb2, -4.0, tcols, ALU.mult, ALU.add)
        nc.vector.tensor_scalar(sb1, st2, 2.0, None, ALU.is_ge)
        nc.vector.scalar_tensor_tensor(sb0, sb1, -2.0, st2, ALU.mult, ALU.add)
        cb = lambda off, w: csts[
            :, coff + off : coff + off + w
        ].unsqueeze(1).broadcast_to((PARTS, G, w))
        bc = lambda b, w: b.unsqueeze(2).broadcast_to((PARTS, G, w))
        nc.vector.tensor_copy(ssel, cb(0, 4))
        nc.vector.copy_predicated(ssel, bc(sb2, 4), cb(4, 4))
        nc.vector.copy_predicated(ssel[:, :, 0:2], bc(sb1, 2), ssel[:, :, 2:4])
        nc.vector.copy_predicated(ssel[:, :, 0:1], bc(sb0, 1), ssel[:, :, 1:2])
        nc.vector.tensor_reduce(
            egp[:, dst_col : dst_col + 1],
            ssel[:, :, 0:1].rearrange("p g c -> p (g c)"),
            mybir.AxisListType.X, ALU.add,
        )

    # ---- prologue: chunk 0 fully staged ----
    eb_cur = load_chunk(0)
    xb_cur = exp_chunk(eb_cur)
    xt_cur = new_xt()
    for sl in range(sc):
        build_xt_step(xb_cur, xt_cur, sl)

    # two independent half-chains (batches split along the free dim) so the
    # PE matmul of one half overlaps the VectorE multiply of the other
    H = PARTS // 2
    PTh = [None, None]
    for h in range(2):
        PTx = state.tile([GJ, H], fp32, tag=f"PT{h}")
        nc.vector.tensor_scalar_mul(
            PTx, xt_cur[:, h * H : h * H + H], selz[:, 16:17]
        )
        PTh[h] = PTx

    kre = 0
    ebs = {0: eb_cur}
    for c in range(nchunk):
        have_next = c + 1 < nchunk
        if have_next:
            ebs[c + 1] = load_chunk(c + 1)
            xb_next = exp_chunk(ebs[c + 1])
            xt_next = new_xt()
        s_lo = c * sc
        if c == 0 and have_next:
            build_xt_step(xb_next, xt_next, 0)  # s-loop below skips s=0
        for s in range(max(s_lo, 1), s_lo + sc):
            sl = s - s_lo
            if s % RENORM == 0:
                mgs = []
                for h in range(2):
                    mg = prp.tile([G, H], fp32, tag=f"mg{h}")
                    nc.tensor.matmul(
                        mg, selz[:, 0:G], PTh[h], start=True, stop=True
                    )
                    nc.scalar.copy(
                        out=mlog[:, kre, h * H : h * H + H], in_=mg
                    )
                    mgs.append(mg)
                for h in range(2):
                    rinv = state.tile([G, H], fp32, tag=f"rinv{h}")
                    nc.vector.reciprocal(rinv, mgs[h])
                    repm = prp.tile([GJ, H], fp32, tag=f"repm{h}")
                    nc.tensor.matmul(repm, rept, rinv, start=True, stop=True)
                    PTn = state.tile([GJ, H], fp32, tag=f"PT{h}")
                    nc.vector.tensor_mul(PTn, PTh[h], repm)
                    PTh[h] = PTn
                kre += 1
            qTs = []
            for h in range(2):
                qT = pqp.tile([GJ, H], fp32, tag=f"qT{h}")
                nc.tensor.matmul(qT, bdt, PTh[h], start=True, stop=True)
                qTs.append(qT)
            for h in range(2):
                PTn = state.tile([GJ, H], fp32, tag=f"PT{h}")
                nc.vector.tensor_mul(
                    PTn, qTs[h], xt_cur[:, sl * PARTS + h * H : sl * PARTS + h * H + H]
                )
                PTh[h] = PTn
            if have_next:
                build_xt_step(xb_next, xt_next, sl)
        egold_chunk(c, ebs[c])
        del ebs[c]
        if have_next:
            xb_cur, xt_cur = xb_next, xt_next

    # ---- final combine ----
    for h in range(2):
        zf = prp.tile([G, H], fp32, tag=f"mg{h}")
        nc.tensor.matmul(zf, selz[:, G : 2 * G], PTh[h], start=True, stop=True)
        nc.scalar.copy(out=mlog[:, n_renorm, h * H : h * H + H], in_=zf)
    lnm = singles.tile([G, n_renorm + 1, PARTS], fp32)
    nc.scalar.activation(
        out=lnm.rearrange("p k b -> p (k b)"),
        in_=mlog.rearrange("p k b -> p (k b)"),
        func=ACTF.Ln,
        scale=float(2.0**-32),
    )
    denb = singles.tile([G, PARTS], fp32)
    nc.vector.tensor_reduce(
        denb, lnm.rearrange("p k b -> p b k"), mybir.AxisListType.X, ALU.add
    )
    nc.sync.dma_start(out=d_ap, in_=denb)

    sel8(nchunk, 0, 16)
    sel8(nchunk + 1, (s_len - 1) * G, 24)
    nc.vector.tensor_reduce(ou[:, 1:2], egp, mybir.AxisListType.X, ALU.add)
    nc.sync.dma_start(out=o_ap, in_=ou)

    for pool in (prp, pqp, ptp, bitp, state, xpool, epool, singles):
        pool.release()


def make_v2_consts(start, end, trans):
    ET = np.exp(trans).astype(np.float32)  # [i, j]
    bd = np.zeros((56, 56), np.float32)
    for g in range(8):
        bd[g * 7 : (g + 1) * 7, g * 7 : (g + 1) * 7] = ET
    selz = np.zeros((56, 17), np.float32)
    rep = np.zeros((8, 56), np.float32)
    for g in range(8):
        for j in range(7):
            selz[g * 7 + j, g] = 1.0
            selz[g * 7 + j, 8 + g] = np.exp(end[j])
            selz[g * 7 + j, 16] = np.exp(start[j])
            rep[g, g * 7 + j] = 1.0
    return bd, selz, rep


_cache = {}


def get_compiled(s_len=S, bs=BS, sc=SC, variant=2):
    key = (s_len, bs, sc, variant)
    if key in _cache:
        return _cache[key]
    import concourse.bacc as bacc
    import concourse.mybir as mybir
    import concourse.tile as tile

    nc = bacc.Bacc(
        "TRN2", target_bir_lowering=False, debug=False, num_devices=NCORES
    )
    fp32 = mybir.dt.float32
    G = bs // PARTS
    e_d = nc.dram_tensor("e", [s_len, bs, T], fp32, kind="ExternalInput").ap()
    tg_d = nc.dram_tensor("tg", [PARTS, s_len * G], fp32, kind="ExternalInput").ap()
    cst_d = nc.dram_tensor("cst", [1, 81], fp32, kind="ExternalInput").ap()
    o_d = nc.dram_tensor("o", [PARTS, 2], fp32, kind="ExternalOutput").ap()
    if variant == 2:
        bd_d = nc.dram_tensor("bd", [56, 56], fp32, kind="ExternalInput").ap()
        selz_d = nc.dram_tensor("selz", [56, 17], fp32, kind="ExternalInput").ap()
        rep_d = nc.dram_tensor("rep", [8, 56], fp32, kind="ExternalInput").ap()
        d_d = nc.dram_tensor("d", [G, PARTS], fp32, kind="ExternalOutput").ap()
        with tile.TileContext(nc) as tc:
            build_body2(
                tc, o_d, d_d, e_d, tg_d, cst_d, bd_d, selz_d, rep_d,
                s_len=s_len, bs=bs, sc=sc,
            )
    else:
        with tile.TileContext(nc) as tc:
            build_body(tc, o_d, e_d, tg_d, cst_d, s_len=s_len, bs=bs, sc=sc)
    nc.compile()
    _cache[key] = nc
    return nc


def make_consts(start, end, trans):
    cst = np.zeros((1, 81), np.float32)
    cst[0, 0:7] = np.exp(start)
    cst[0, 8:15] = np.exp(end)
    cst[0, 16:23] = start
    cst[0, 24:31] = end
    cst[0, 32:81] = np.exp(trans).T.ravel()  # ET[j, i] = exp(trans[i, j])
    return cst


def _numpy_fallback(emissions, start, end, trans, tags, mask):
    maskf = mask.astype(np.float64)
    e = emissions.astype(np.float64)
    s_len, batch = tags.shape
    emit = np.take_along_axis(e, tags[:, :, None], axis=2)[..., 0]
    trans_sc = trans[tags[:-1], tags[1:]].astype(np.float64)
    num = start[tags[0]].astype(np.float64) + emit[0]
    num = num + ((trans_sc + emit[1:]) * maskf[1:]).sum(axis=0)
    seq_ends = mask.astype(np.int64).sum(axis=0) - 1
    last_tags = tags[seq_ends, np.arange(batch)]
    num = num + end[last_tags]
    score = start[None, :] + e[0]
    for i in range(1, s_len):
        nxt = score[:, :, None] + trans[None] + e[i][:, None, :]
        mx = nxt.max(axis=1)
        nxt = mx + np.log(np.exp(nxt - mx[:, None, :]).sum(axis=1))
        score = np.where(mask[i][:, None], nxt, score)
    mx = (score + end[None, :]).max(axis=1)
    denom = mx + np.log(np.exp(score + end[None, :] - mx[:, None]).sum(axis=1))
    return np.float32((num - denom).sum())


def kernel(emissions, start_transitions, end_transitions, transitions, tags, mask):
    global LAST_EXEC_NS
    emissions = np.asarray(emissions, np.float32)
    start = np.asarray(start_transitions, np.float32)
    end = np.asarray(end_transitions, np.float32)
    trans = np.asarray(transitions, np.float32)
    tags = np.asarray(tags)
    mask_np = np.asarray(mask)

    if not mask_np.all():
        return _numpy_fallback(
            emissions, start, end, trans, tags.astype(np.int64), mask_np
        )

    from concourse import bass_utils

    variant = 2
    nc = get_compiled(variant=variant)
    cst = make_consts(start, end, trans)
    tags32 = tags.astype(np.int32)
    in_maps = []
    G = BS // PARTS
    if variant == 2:
        bd, selz, rep = make_v2_consts(start, end, trans)
    for c in range(NCORES):
        sl = slice(c * BS, (c + 1) * BS)
        e_sh = np.ascontiguousarray(emissions[:, sl, :])
        tgc = (
            tags32[:, sl]
            .reshape(S, G, PARTS)
            .transpose(2, 0, 1)
            .reshape(PARTS, S * G)
            .astype(np.float32)
        )
        m = {"e": e_sh, "tg": np.ascontiguousarray(tgc), "cst": cst}
        if variant == 2:
            m.update({"bd": bd, "selz": selz, "rep": rep})
        in_maps.append(m)

    trace = TRACE
    if trace:
        try:
            from antenv.axon_hooks import get_axon_ntff_profile_hook  # noqa: F401
        except ImportError:
            trace = False
    res = bass_utils.run_bass_kernel_spmd(
        nc, in_maps, core_ids=list(range(NCORES)), trace=trace
    )
    LAST_EXEC_NS = res.exec_time_ns

    total = 0.0
    for c in range(NCORES):
        o = res.results[c]["o"].astype(np.float64)
        total += o[:, 1].sum() - o[:, 0].sum()
        if variant == 2:
            total -= res.results[c]["d"].astype(np.float64).sum()
    # Ln-scale correction: device computed ln(m * 2^-32) per mlog slot
    n_renorm = (S - 1) // RENORM
    total -= B * (n_renorm + 1) * 32.0 * np.log(2.0)

    # host part: sum_s trans[t_s, t_{s+1}] via 49-bin histogram
    codes = (7 * tags32[:-1] + tags32[1:]).ravel()
    cnt = np.bincount(codes, minlength=49).astype(np.float64)
    total += float(cnt @ trans.astype(np.float64).ravel())
    return np.float32(total)
